# revision 29
# baseline (speedup 1.0000x reference)
"""Trainium2 Bass kernel for BSplineNN: cubic B-spline evaluation.

out[b, c] = sum_i coefficients[b, i, c] * N_{i,3}(x_b),  x_b = inpce[b, 0]

Key property exploited: a cubic B-spline basis at a single point has at most
4 non-zero entries (rows i0..i0+3 with i0 = clamp(searchsorted(t, x) - 4, 0, 60)).
So instead of reading all 64 coefficient rows per batch (268 MB total), we:
  1. compute the knot-interval index on-chip from the (small) knots tensor,
  2. indirect-DMA-gather only the 4 relevant coefficient rows (4 KB/batch)
     and the 8 relevant knots per batch,
  3. run the Cox-de Boor recurrence on the 8-knot window (sizes 7->6->5->4),
     giving exactly the 4 non-zero basis weights,
  4. weighted-sum the 4 gathered rows.

Sharding: pure data parallel, batch dim split across 8 cores (512 each).
Within a core, batch b = 4*p + g (p = partition 0..127, g = group 0..3).
"""

import numpy as np

import concourse.bacc as bacc
import concourse.bass as bass
import concourse.mybir as mybir
import concourse.tile as tile
from concourse.bass_utils import run_bass_kernel_spmd

B, N, C, T = 4096, 64, 256, 68   # batch, coef rows, channels, knots
K = 3                            # cubic
NCORES = 8
BC = B // NCORES                 # 512 batches per core
P = 128                          # partitions
G = BC // P                      # 4 batch-groups per partition
WROWS = K + 1                    # 4 gathered coef rows per batch
WKNOTS = 2 * K + 2               # 8 gathered knots per batch
F32 = mybir.dt.float32
I32 = mybir.dt.int32


def _emit(tc, nc, coef, knots, inpce, out):
    with tc.tile_pool(name="sb", bufs=1) as sb:
        # ---- load knots + x (layout b = 4p + g) ----
        kt = sb.tile([P, G, T], F32)
        nc.sync.dma_start(out=kt[:], in_=knots.rearrange("(p g) t -> p g t", g=G))
        xt = sb.tile([P, G], F32)
        nc.scalar.dma_start(out=xt[:], in_=inpce.rearrange("(p g) o -> p (g o)", g=G))

        # ---- interval index: cnt = #{j : t[j] <= x},  i0 = clamp(cnt-4, 0, 60) ----
        ind = sb.tile([P, G, T], F32)
        nc.vector.tensor_tensor(out=ind[:], in0=xt[:].to_broadcast([P, G, T]),
                                in1=kt[:], op=mybir.AluOpType.is_ge)
        cnt = sb.tile([P, G], F32)
        nc.vector.reduce_sum(out=cnt[:], in_=ind[:], axis=mybir.AxisListType.X)
        i0f = sb.tile([P, G], F32)
        nc.vector.tensor_scalar(out=i0f[:], in0=cnt[:], scalar1=4.0, scalar2=0.0,
                                op0=mybir.AluOpType.subtract, op1=mybir.AluOpType.max)
        nc.vector.tensor_scalar_min(out=i0f[:], in0=i0f[:], scalar1=float(N - WROWS))
        i0i = sb.tile([P, G], I32)
        nc.vector.tensor_copy(out=i0i[:], in_=i0f[:])

        # ---- gather indices ----
        # coef rows: flat row index into [BC*N, C] = (4p+g)*N + i0
        bi = sb.tile([P, G], I32)
        nc.gpsimd.iota(out=bi[:], pattern=[[N, G]], base=0, channel_multiplier=N * G)
        gidx = sb.tile([P, G], I32)
        nc.vector.tensor_tensor(out=gidx[:], in0=bi[:], in1=i0i[:],
                                op=mybir.AluOpType.add)
        # knot window: flat element index into [BC*T] = (4p+g)*T + i0
        bik = sb.tile([P, G], I32)
        nc.gpsimd.iota(out=bik[:], pattern=[[T, G]], base=0, channel_multiplier=T * G)
        kidx = sb.tile([P, G], I32)
        nc.vector.tensor_tensor(out=kidx[:], in0=bik[:], in1=i0i[:],
                                op=mybir.AluOpType.add)

        # HW indirect DMA consumes ONE index per partition and gathers
        # out_free_size contiguous elements per partition, so issue one
        # gather per batch-group g with a [P, 1] index slice.
        # SWDGE emission order (the serial Pool engine AND the single SWDGE
        # DMA queue are serial resources): all four tiny knot-window gathers
        # first — their data jumps the queue ahead of the 512 KB coef blocks,
        # so the basis is done early and each group's contraction follows its
        # own coef block as it streams in.
        kw = sb.tile([P, G, WKNOTS], F32)
        gt = sb.tile([P, G, WROWS * C], F32)
        order = []
        for g in range(G):
            order.append(nc.gpsimd.indirect_dma_start(
                out=kw[:][:, g, :], out_offset=None,
                in_=knots.rearrange("b (t o) -> (b t) o", o=1),
                in_offset=bass.IndirectOffsetOnAxis(
                    ap=kidx[:][:, g:g + 1], axis=0)))
        for g in range(G):
            order.append(nc.gpsimd.indirect_dma_start(
                out=gt[:][:, g, :], out_offset=None,
                in_=coef.rearrange("b n c -> (b n) c"),
                in_offset=bass.IndirectOffsetOnAxis(
                    ap=gidx[:][:, g:g + 1], axis=0)))
        for a, b in zip(order[1:], order):
            tile.add_dep_helper(a.ins, b.ins, sync=False,
                                reason="SWDGE emission order")

        # ---- windowed Cox-de Boor on kw, in two independent halves ----
        # Uses w1[i] = U[i], w2[i] = 1 - U[i+1] with U[j] = (x-t[j])/(t[j+kk]-t[j]):
        #   Bnew[i] = U[i]*B[i] + B[i+1] - U[i+1]*B[i+1]   (7 ops per level)
        indw = sb.tile([P, G, WKNOTS], F32)
        xmt = sb.tile([P, G, WKNOTS], F32)
        levels = [sb.tile([P, G, WKNOTS - 1 - kk], F32, name=f"lvl{kk}")
                  for kk in range(K + 1)]
        HG = 2  # groups per half
        for h in (0, 1):
            gs = slice(HG * h, HG * h + HG)
            kwh = kw[:][:, gs, :]
            xb8 = xt[:][:, gs].to_broadcast([P, HG, WKNOTS])
            nc.vector.tensor_tensor(out=indw[:][:, gs, :], in0=xb8, in1=kwh,
                                    op=mybir.AluOpType.is_ge)
            nc.vector.tensor_tensor(out=xmt[:][:, gs, :], in0=xb8, in1=kwh,
                                    op=mybir.AluOpType.subtract)
            nc.vector.tensor_tensor(
                out=levels[0][:][:, gs, :],
                in0=indw[:][:, gs, 0:WKNOTS - 1],
                in1=indw[:][:, gs, 1:WKNOTS], op=mybir.AluOpType.subtract)
            for kk in range(1, K + 1):
                L = WKNOTS - 1 - kk
                prev = levels[kk - 1][:][:, gs, :]
                d1 = sb.tile([P, HG, L + 1], F32, tag=f"d1_{kk}_{h}")
                u1 = sb.tile([P, HG, L + 1], F32, tag=f"u1_{kk}_{h}")
                a1 = sb.tile([P, HG, L], F32, tag=f"a1_{kk}_{h}")
                t2 = sb.tile([P, HG, L], F32, tag=f"t2_{kk}_{h}")
                nc.vector.tensor_tensor(out=d1[:], in0=kwh[:, :, kk:kk + L + 1],
                                        in1=kwh[:, :, 0:L + 1],
                                        op=mybir.AluOpType.subtract)
                nc.vector.reciprocal(out=u1[:], in_=d1[:])
                nc.vector.tensor_tensor(out=u1[:], in0=xmt[:][:, gs, 0:L + 1],
                                        in1=u1[:], op=mybir.AluOpType.mult)
                nc.vector.tensor_tensor(out=a1[:], in0=u1[:][:, :, 0:L],
                                        in1=prev[:, :, 0:L],
                                        op=mybir.AluOpType.mult)
                nc.vector.tensor_tensor(out=t2[:], in0=u1[:][:, :, 1:L + 1],
                                        in1=prev[:, :, 1:1 + L],
                                        op=mybir.AluOpType.mult)
                nc.vector.tensor_tensor(out=t2[:], in0=prev[:, :, 1:1 + L],
                                        in1=t2[:], op=mybir.AluOpType.subtract)
                nc.vector.tensor_tensor(out=levels[kk][:][:, gs, :], in0=a1[:],
                                        in1=t2[:], op=mybir.AluOpType.add)
        wts = levels[K]  # [P, G, 4] basis weights for rows i0..i0+3

        # ---- weighted sum of the 4 gathered rows, per group (pipelines with
        # the coef gathers; wts[:, g, d] is a [P,1] per-partition scalar).
        # d=0 multiply runs on the otherwise-idle ACT engine; the stt chain
        # stays on DVE; each group's result is stored as soon as it's done
        # (alternating the two HWDGE rings). ----
        gtv = gt[:].rearrange("p g (d c) -> p g d c", d=WROWS)
        outv = out.rearrange("(p g) c -> p g c", g=G)
        acc = sb.tile([P, G, C], F32)
        for g in range(G):
            nc.vector.tensor_scalar_mul(out=acc[:][:, g, :], in0=gtv[:, g, 0, :],
                                        scalar1=wts[:][:, g, 0:1])
            for d in range(1, WROWS):
                nc.vector.scalar_tensor_tensor(
                    out=acc[:][:, g, :], in0=gtv[:, g, d, :],
                    scalar=wts[:][:, g, d:d + 1], in1=acc[:][:, g, :],
                    op0=mybir.AluOpType.mult, op1=mybir.AluOpType.add)
            eng = nc.sync if g % 2 == 0 else nc.scalar
            eng.dma_start(out=outv[:, g, :], in_=acc[:][:, g, :])


def build_nc(reps=1):
    nc = bacc.Bacc("TRN2", target_bir_lowering=False, debug=False,
                   num_devices=NCORES)
    coef = nc.dram_tensor("coefficients", [BC, N, C], F32, kind="ExternalInput")
    knots = nc.dram_tensor("knots", [BC, T], F32, kind="ExternalInput")
    inpce = nc.dram_tensor("inpce", [BC, 1], F32, kind="ExternalInput")
    out = nc.dram_tensor("out", [BC, C], F32, kind="ExternalOutput")
    with tile.TileContext(nc) as tc:
        for _ in range(reps):
            _emit(tc, nc, coef.ap(), knots.ap(), inpce.ap(), out.ap())
    nc.compile()
    return nc


def build_nc_loop(trip):
    """Kernel body wrapped in a hardware For_i loop — for benchmarking only."""
    nc = bacc.Bacc("TRN2", target_bir_lowering=False, debug=False,
                   num_devices=NCORES)
    coef = nc.dram_tensor("coefficients", [BC, N, C], F32, kind="ExternalInput")
    knots = nc.dram_tensor("knots", [BC, T], F32, kind="ExternalInput")
    inpce = nc.dram_tensor("inpce", [BC, 1], F32, kind="ExternalInput")
    out = nc.dram_tensor("out", [BC, C], F32, kind="ExternalOutput")
    with tile.TileContext(nc) as tc:
        with tc.For_i(0, trip, 1):
            _emit(tc, nc, coef.ap(), knots.ap(), inpce.ap(), out.ap())
    nc.compile()
    return nc


_NC_CACHE = None


def kernel(coefficients, knots, inpce, **run_kwargs):
    global _NC_CACHE
    if _NC_CACHE is None:
        _NC_CACHE = build_nc()
    nc = _NC_CACHE
    coefficients = np.ascontiguousarray(coefficients, dtype=np.float32)
    knots = np.ascontiguousarray(knots, dtype=np.float32)
    inpce = np.ascontiguousarray(inpce, dtype=np.float32)
    in_maps = []
    for k in range(NCORES):
        s = slice(k * BC, (k + 1) * BC)
        in_maps.append({"coefficients": coefficients[s],
                        "knots": knots[s],
                        "inpce": inpce[s]})
    res = run_bass_kernel_spmd(nc, in_maps, core_ids=list(range(NCORES)),
                               **run_kwargs)
    out = np.concatenate([res.results[k]["out"] for k in range(NCORES)], axis=0)
    if run_kwargs:
        return out, res
    return out


# revision 31
# speedup vs baseline: 1.7252x; 1.7252x over previous
"""Trainium2 Bass kernel for BSplineNN: cubic B-spline evaluation.

out[b, c] = sum_i coefficients[b, i, c] * N_{i,3}(x_b),  x_b = inpce[b, 0]

Key property exploited: a cubic B-spline basis at a single point has at most
4 non-zero entries (rows i0..i0+3 with i0 = clamp(searchsorted(t, x) - 4, 0, 60)
= #{j in [4, 64): t[j] <= x} since the knots are sorted).
So instead of reading all 64 coefficient rows per batch (268 MB total), we:
  1. compute the knot-interval index on-chip from the (small) knots tensor,
  2. indirect-DMA-gather only the 4 relevant coefficient rows (4 KB/batch)
     and the 8 relevant knots per batch,
  3. run the Cox-de Boor recurrence on the 8-knot window (sizes 7->6->5->4),
     giving exactly the 4 non-zero basis weights,
  4. weighted-sum the 4 gathered rows.

Sharding: pure data parallel, batch dim split across 8 cores (512 each).
Within a core, batch b = 4*p + g (p = partition 0..127, g = group 0..3).
"""

import numpy as np

import concourse.bacc as bacc
import concourse.bass as bass
import concourse.mybir as mybir
import concourse.tile as tile
from concourse.bass_utils import run_bass_kernel_spmd

B, N, C, T = 4096, 64, 256, 68   # batch, coef rows, channels, knots
K = 3                            # cubic
NCORES = 8
BC = B // NCORES                 # 512 batches per core
P = 128                          # partitions
G = BC // P                      # 4 batch-groups per partition
WROWS = K + 1                    # 4 gathered coef rows per batch
WKNOTS = 2 * K + 2               # 8 gathered knots per batch
F32 = mybir.dt.float32
I32 = mybir.dt.int32


def _emit(tc, nc, coef, knots, inpce, out):
    with tc.tile_pool(name="sb", bufs=1) as sb:
        # ---- load knots + x (layout b = 4p + g) ----
        kt = sb.tile([P, G, T], F32)
        nc.sync.dma_start(out=kt[:], in_=knots.rearrange("(p g) t -> p g t", g=G))
        xt = sb.tile([P, G], F32)
        nc.scalar.dma_start(out=xt[:], in_=inpce.rearrange("(p g) o -> p (g o)", g=G))

        # ---- interval index ----
        # i0 = clamp(#{j in [0,68): t[j] <= x} - 4, 0, 60) is identically
        # #{j in [4,64): t[j] <= x} (knots sorted), so compare only the middle
        # 60 knots and skip the clamp ops entirely.
        ind = sb.tile([P, G, N - WROWS], F32)
        nc.vector.tensor_tensor(out=ind[:],
                                in0=xt[:].to_broadcast([P, G, N - WROWS]),
                                in1=kt[:][:, :, WROWS:N],
                                op=mybir.AluOpType.is_ge)
        i0f = sb.tile([P, G], F32)
        nc.vector.reduce_sum(out=i0f[:], in_=ind[:], axis=mybir.AxisListType.X)
        i0i = sb.tile([P, G], I32)
        nc.vector.tensor_copy(out=i0i[:], in_=i0f[:])

        # ---- gather indices ----
        # coef rows: flat row index into [BC*N, C] = (4p+g)*N + i0
        bi = sb.tile([P, G], I32)
        nc.gpsimd.iota(out=bi[:], pattern=[[N, G]], base=0, channel_multiplier=N * G)
        gidx = sb.tile([P, G], I32)
        nc.vector.tensor_tensor(out=gidx[:], in0=bi[:], in1=i0i[:],
                                op=mybir.AluOpType.add)
        # knot window: flat element index into [BC*T] = (4p+g)*T + i0
        bik = sb.tile([P, G], I32)
        nc.gpsimd.iota(out=bik[:], pattern=[[T, G]], base=0, channel_multiplier=T * G)
        kidx = sb.tile([P, G], I32)
        nc.vector.tensor_tensor(out=kidx[:], in0=bik[:], in1=i0i[:],
                                op=mybir.AluOpType.add)

        # HW indirect DMA consumes ONE index per partition and gathers
        # out_free_size contiguous elements per partition, so issue one
        # gather per batch-group g with a [P, 1] index slice.
        # SWDGE emission order (the serial Pool engine AND the single SWDGE
        # DMA queue are serial resources): all four tiny knot-window gathers
        # first — their data jumps the queue ahead of the 512 KB coef blocks,
        # so the basis is done early and each group's contraction follows its
        # own coef block as it streams in.
        kw = sb.tile([P, G, WKNOTS], F32)
        gt = sb.tile([P, G, WROWS * C], F32)
        order = []
        for g in range(G):
            order.append(nc.gpsimd.indirect_dma_start(
                out=kw[:][:, g, :], out_offset=None,
                in_=knots.rearrange("b (t o) -> (b t) o", o=1),
                in_offset=bass.IndirectOffsetOnAxis(
                    ap=kidx[:][:, g:g + 1], axis=0)))
        for g in range(G):
            order.append(nc.gpsimd.indirect_dma_start(
                out=gt[:][:, g, :], out_offset=None,
                in_=coef.rearrange("b n c -> (b n) c"),
                in_offset=bass.IndirectOffsetOnAxis(
                    ap=gidx[:][:, g:g + 1], axis=0)))
        for a, b in zip(order[1:], order):
            tile.add_dep_helper(a.ins, b.ins, sync=False,
                                reason="SWDGE emission order")

        # ---- windowed Cox-de Boor on kw, in two independent halves ----
        # Uses w1[i] = U[i], w2[i] = 1 - U[i+1] with U[j] = (x-t[j])/(t[j+kk]-t[j]):
        #   Bnew[i] = U[i]*B[i] + B[i+1] - U[i+1]*B[i+1]   (7 ops per level)
        indw = sb.tile([P, G, WKNOTS], F32)
        xmt = sb.tile([P, G, WKNOTS], F32)
        levels = [sb.tile([P, G, WKNOTS - 1 - kk], F32, name=f"lvl{kk}")
                  for kk in range(K + 1)]
        HG = 2  # groups per half
        for h in (0, 1):
            gs = slice(HG * h, HG * h + HG)
            kwh = kw[:][:, gs, :]
            xb8 = xt[:][:, gs].to_broadcast([P, HG, WKNOTS])
            nc.vector.tensor_tensor(out=indw[:][:, gs, :], in0=xb8, in1=kwh,
                                    op=mybir.AluOpType.is_ge)
            nc.vector.tensor_tensor(out=xmt[:][:, gs, :], in0=xb8, in1=kwh,
                                    op=mybir.AluOpType.subtract)
            nc.vector.tensor_tensor(
                out=levels[0][:][:, gs, :],
                in0=indw[:][:, gs, 0:WKNOTS - 1],
                in1=indw[:][:, gs, 1:WKNOTS], op=mybir.AluOpType.subtract)
            for kk in range(1, K + 1):
                L = WKNOTS - 1 - kk
                prev = levels[kk - 1][:][:, gs, :]
                d1 = sb.tile([P, HG, L + 1], F32, tag=f"d1_{kk}_{h}")
                u1 = sb.tile([P, HG, L + 1], F32, tag=f"u1_{kk}_{h}")
                a1 = sb.tile([P, HG, L], F32, tag=f"a1_{kk}_{h}")
                t2 = sb.tile([P, HG, L], F32, tag=f"t2_{kk}_{h}")
                nc.vector.tensor_tensor(out=d1[:], in0=kwh[:, :, kk:kk + L + 1],
                                        in1=kwh[:, :, 0:L + 1],
                                        op=mybir.AluOpType.subtract)
                nc.vector.reciprocal(out=u1[:], in_=d1[:])
                nc.vector.tensor_tensor(out=u1[:], in0=xmt[:][:, gs, 0:L + 1],
                                        in1=u1[:], op=mybir.AluOpType.mult)
                nc.vector.tensor_tensor(out=a1[:], in0=u1[:][:, :, 0:L],
                                        in1=prev[:, :, 0:L],
                                        op=mybir.AluOpType.mult)
                nc.vector.tensor_tensor(out=t2[:], in0=u1[:][:, :, 1:L + 1],
                                        in1=prev[:, :, 1:1 + L],
                                        op=mybir.AluOpType.mult)
                nc.vector.tensor_tensor(out=t2[:], in0=prev[:, :, 1:1 + L],
                                        in1=t2[:], op=mybir.AluOpType.subtract)
                nc.vector.tensor_tensor(out=levels[kk][:][:, gs, :], in0=a1[:],
                                        in1=t2[:], op=mybir.AluOpType.add)
        wts = levels[K]  # [P, G, 4] basis weights for rows i0..i0+3

        # ---- weighted sum of the 4 gathered rows, per group (pipelines with
        # the coef gathers; wts[:, g, d] is a [P,1] per-partition scalar).
        # d=0 multiply runs on the otherwise-idle ACT engine; the stt chain
        # stays on DVE; each group's result is stored as soon as it's done
        # (alternating the two HWDGE rings). ----
        gtv = gt[:].rearrange("p g (d c) -> p g d c", d=WROWS)
        outv = out.rearrange("(p g) c -> p g c", g=G)
        acc = sb.tile([P, G, C], F32)
        for g in range(G):
            nc.vector.tensor_scalar_mul(out=acc[:][:, g, :], in0=gtv[:, g, 0, :],
                                        scalar1=wts[:][:, g, 0:1])
            for d in range(1, WROWS):
                nc.vector.scalar_tensor_tensor(
                    out=acc[:][:, g, :], in0=gtv[:, g, d, :],
                    scalar=wts[:][:, g, d:d + 1], in1=acc[:][:, g, :],
                    op0=mybir.AluOpType.mult, op1=mybir.AluOpType.add)
            eng = nc.sync if g % 2 == 0 else nc.scalar
            eng.dma_start(out=outv[:, g, :], in_=acc[:][:, g, :])


def build_nc(reps=1):
    nc = bacc.Bacc("TRN2", target_bir_lowering=False, debug=False,
                   num_devices=NCORES)
    coef = nc.dram_tensor("coefficients", [BC, N, C], F32, kind="ExternalInput")
    knots = nc.dram_tensor("knots", [BC, T], F32, kind="ExternalInput")
    inpce = nc.dram_tensor("inpce", [BC, 1], F32, kind="ExternalInput")
    out = nc.dram_tensor("out", [BC, C], F32, kind="ExternalOutput")
    with tile.TileContext(nc) as tc:
        for _ in range(reps):
            _emit(tc, nc, coef.ap(), knots.ap(), inpce.ap(), out.ap())
    nc.compile()
    return nc


def build_nc_loop(trip):
    """Kernel body wrapped in a hardware For_i loop — for benchmarking only."""
    nc = bacc.Bacc("TRN2", target_bir_lowering=False, debug=False,
                   num_devices=NCORES)
    coef = nc.dram_tensor("coefficients", [BC, N, C], F32, kind="ExternalInput")
    knots = nc.dram_tensor("knots", [BC, T], F32, kind="ExternalInput")
    inpce = nc.dram_tensor("inpce", [BC, 1], F32, kind="ExternalInput")
    out = nc.dram_tensor("out", [BC, C], F32, kind="ExternalOutput")
    with tile.TileContext(nc) as tc:
        with tc.For_i(0, trip, 1):
            _emit(tc, nc, coef.ap(), knots.ap(), inpce.ap(), out.ap())
    nc.compile()
    return nc


_NC_CACHE = None


def kernel(coefficients, knots, inpce, **run_kwargs):
    global _NC_CACHE
    if _NC_CACHE is None:
        _NC_CACHE = build_nc()
    nc = _NC_CACHE
    coefficients = np.ascontiguousarray(coefficients, dtype=np.float32)
    knots = np.ascontiguousarray(knots, dtype=np.float32)
    inpce = np.ascontiguousarray(inpce, dtype=np.float32)
    in_maps = []
    for k in range(NCORES):
        s = slice(k * BC, (k + 1) * BC)
        in_maps.append({"coefficients": coefficients[s],
                        "knots": knots[s],
                        "inpce": inpce[s]})
    res = run_bass_kernel_spmd(nc, in_maps, core_ids=list(range(NCORES)),
                               **run_kwargs)
    out = np.concatenate([res.results[k]["out"] for k in range(NCORES)], axis=0)
    if run_kwargs:
        return out, res
    return out
